# revision 1
# baseline (speedup 1.0000x reference)
"""DenseContrastiveLoss forward on 8 Trainium2 NeuronCores.

Reference math:
    C = concat([f1.reshape(B,-1), f2.reshape(B,-1)])          # (512, 65536)
    G = C @ C.T ; sq[i] = ||C_i||^2
    A[i,j] = -0.01*(sq[i] + sq[j] - 2 G[i,j])
    loss = mean_i -(A[i,p(i)] - max_j A[i,j]
                    - log(sum_j exp(A-max)*offdiag + 1e-10))

The per-row term -0.01*sq[i] is constant along each row: it cancels in
(A - rowmax) and in (A[partner] - rowmax), so the device works with
B[i,j] = 0.02*G[i,j] - 0.01*sq[j] only. sq is the cheap part (one pass over
the inputs) and is computed on the host and shipped as a tiny replicated
input; the 34 GFLOP Gram matrix and the softmax rows run on device.

Sharding: K-parallel. Core c holds ct = C[:, shard_c].T (8192x512, fp8-e4m3,
pre-swizzled to partition-major) and accumulates a partial 512x512 Gram in
PSUM with 128 DoubleRow matmuls (K=256 each). The partial grams (minus each
core's host-known fp8 diagonal, cast fp16 — kills both the fp16 overflow and
the fp8 sum(r^2) diagonal bias) are combined by an 8-core ReduceScatter that
hands core c rows [64c, 64c+64); a tiny AllGather issued at kernel start
soaks up the runtime's global-comm barrier so the ReduceScatter runs hot.
Each core then runs the softmax-loss row epilogue on its 64 rows;
rank-dependent row/partner masks arrive as per-core input data so the SPMD
program itself is rank-independent. Each core emits per-row losses; the host
sums 512 values and divides by N (the mean-reduction unshard step).
"""

import sys

if "/opt/trn_rl_repo" not in sys.path:
    sys.path.insert(0, "/opt/trn_rl_repo")

import ml_dtypes
import numpy as np

import concourse.bass as bass  # noqa: F401
import concourse.mybir as mybir
import concourse.tile as tile
from concourse import bacc
from concourse.bass import ts
from concourse.bass_utils import run_bass_kernel_spmd

N_CORES = 8
B = 256
N = 2 * B  # 512 contrast rows
K = 65536  # feature dim (256*16*16)
P = 128
TEMP = 0.01  # TEMPERATURE (== BASE_TEMPERATURE, ratio 1.0)
RPC = N // N_CORES  # rows per core after ReduceScatter (64)


def build_nc(kshard=K // N_CORES, n_cores=N_CORES):
    nc = bacc.Bacc(
        "TRN2",
        target_bir_lowering=False,
        debug=False,
        enable_asserts=False,
        num_devices=n_cores,
    )
    rpc = N // n_cores
    ct_h = nc.dram_tensor("ct", [P, kshard // P, N], mybir.dt.float8e4, kind="ExternalInput")
    sqb_h = nc.dram_tensor("sqb", [rpc, N], mybir.dt.float32, kind="ExternalInput")
    adm_h = nc.dram_tensor("adm", [rpc, N], mybir.dt.float32, kind="ExternalInput")
    pm_h = nc.dram_tensor("pm", [rpc, N], mybir.dt.float32, kind="ExternalInput")
    dsub_h = nc.dram_tensor("dsub", [N // P, P, N], mybir.dt.float32, kind="ExternalInput")
    out_h = nc.dram_tensor("out", [rpc, 1], mybir.dt.float32, kind="ExternalOutput")
    aps = dict(
        ct=ct_h.ap(), sqb=sqb_h.ap(), adm=adm_h.ap(), pm=pm_h.ap(),
        dsub=dsub_h.ap(), out=out_h.ap(),
    )
    with tile.TileContext(nc) as tc:
        _body(tc, nc, aps, kshard, n_cores)
    nc.compile()
    return nc


def _body(tc, nc, aps, kshard, n_cores):
    ct, sqb, adm, pm = aps["ct"], aps["sqb"], aps["adm"], aps["pm"]
    dsub, out = aps["dsub"], aps["out"]
    f32 = mybir.dt.float32
    bf16 = mybir.dt.bfloat16
    f16 = mybir.dt.float16
    rpc = N // n_cores
    MB = N // P  # 4 row-blocks of the 512x512 gram
    CH = 4  # 128-deep k-chunks per DMA tile (512 KiB bf16 DMAs)
    assert kshard % (CH * P) == 0
    NT = kshard // (CH * P)
    X = mybir.AxisListType.X
    add = mybir.AluOpType.add
    mult = mybir.AluOpType.mult
    sub = mybir.AluOpType.subtract
    mx_op = mybir.AluOpType.max
    AF = mybir.ActivationFunctionType

    NCH = kshard // P  # 128-deep k-chunks total (64 at full size)
    # small leading DMA groups so the first matmuls start early
    groups = [2, 6] + [8] * ((NCH - 8) // 8)
    assert sum(groups) == NCH and all(g % 2 == 0 for g in groups)
    f8 = mybir.dt.float8e4
    DR = mybir.MatmulPerfMode.DoubleRow

    with (
        tc.tile_pool(name="ctp", bufs=6) as ctp,
        tc.tile_pool(name="gacc", bufs=1, space="PSUM") as gacc,
        tc.tile_pool(name="sb", bufs=1) as sb,
        tc.tile_pool(name="epp", bufs=1, space="PSUM") as epp,
        tc.tile_pool(name="dram", bufs=1, space="DRAM") as dram,
    ):
        # tiny early collective: soaks up the runtime's global-comm barrier and
        # ncfw cold-start while the gram stream runs, so the ReduceScatter
        # later runs on a hot collective engine (measured 14us vs 27-35 cold)
        warm_in = dram.tile([1, 1], f32)
        warm_out = dram.tile([n_cores, 1], f32)
        wtmp = sb.tile([1, 1], f32, tag="wtmp")
        nc.vector.memset(wtmp[:], 0.0)
        nc.gpsimd.dma_start(warm_in[:], wtmp[:])
        nc.gpsimd.collective_compute(
            "AllGather",
            mybir.AluOpType.bypass,
            replica_groups=[list(range(n_cores))],
            ins=[warm_in.opt()],
            outs=[warm_out.opt()],
        )

        # ---- partial gram over this core's K shard (fp8 DoubleRow: K=256/mm)
        acc = [gacc.tile([P, N], f32, tag=f"acc{m}", name=f"acc{m}") for m in range(MB)]
        o = 0
        for g in groups:
            cts = ctp.tile([P, 8, N], f8, tag="ct")
            nc.sync.dma_start(cts[:, :g, :], ct[:, o : o + g, :])
            for cc in range(0, g, 2):
                for m in range(MB):
                    nc.tensor.matmul(
                        acc[m][:],
                        lhsT=cts[:, cc : cc + 2, ts(m, P)],
                        rhs=cts[:, cc : cc + 2, :],
                        perf_mode=DR,
                        start=(o == 0 and cc == 0),
                        stop=(o + g == NCH and cc == g - 2),
                    )
            o += g

        # ---- (gram - diag(sq)/ncores) -> fp16 -> DRAM, ReduceScatter across cores
        # Subtracting the (host-known) diagonal keeps every entry small enough
        # for fp16 (the raw diagonal ~K overflows fp16 and would dominate its
        # rounding); the exact diagonal is re-added after the scatter.
        dsub_sb = sb.tile([P, MB, N], f32, tag="dsub")
        nc.gpsimd.dma_start(dsub_sb[:], dsub.rearrange("m p j -> p m j"))
        gram_sb = sb.tile([P, MB, N], f16, tag="gram")
        for m in range(MB):
            nc.vector.tensor_tensor(gram_sb[:, m, :], acc[m][:], dsub_sb[:, m, :], sub)
        cc_in = dram.tile([N, N], f16)
        cc_rs = dram.tile([rpc, N], f16)
        nc.sync.dma_start(cc_in.rearrange("(m p) j -> p m j", p=P), gram_sb[:])
        # ReduceScatter sums the partials and hands core c rows [64c, 64c+64)
        nc.gpsimd.collective_compute(
            "ReduceScatter",
            add,
            replica_groups=[list(range(n_cores))],
            ins=[cc_in.opt()],
            outs=[cc_rs.opt()],
        )

        # ---- epilogue on this core's rpc rows ----
        sqb_sb = sb.tile([rpc, N], f32, tag="sqb")
        adm_sb = sb.tile([rpc, N], f32, tag="adm")
        pm_sb = sb.tile([rpc, N], f32, tag="pm")
        nc.gpsimd.dma_start(sqb_sb[:], sqb)
        nc.gpsimd.dma_start(adm_sb[:], adm)
        nc.gpsimd.dma_start(pm_sb[:], pm)
        epsb = sb.tile([rpc, 1], f32, tag="epsb")
        nc.vector.memset(epsb[:], 1.0e-10)

        g = sb.tile([rpc, N], f16, tag="g")
        nc.sync.dma_start(g[:], cc_rs[:])
        # B' = B/0.02 = H + input(-0.5*sq_j + sq diag one-hot); the 0.02 scale
        # is folded into the Exp and the final combine
        tt = sb.tile([rpc, N], f32, tag="tt")
        nc.vector.tensor_scalar_mul(tt[:], g[:], 1.0)
        nc.vector.tensor_tensor(tt[:], tt[:], sqb_sb[:], add)
        mx = sb.tile([rpc, 1], f32, tag="mx")
        nc.vector.reduce_max(mx[:], tt[:], axis=X)
        nmx = sb.tile([rpc, 1], f32, tag="nmx")
        nc.vector.tensor_scalar_mul(nmx[:], mx[:], -2.0 * TEMP)
        # positive-pair logit via per-core one-hot mask
        tp_ = sb.tile([rpc, N], f32, tag="tp")
        nc.vector.tensor_tensor(tp_[:], tt[:], pm_sb[:], mult)
        spos = sb.tile([rpc, 1], f32, tag="spos")
        nc.vector.reduce_sum(spos[:], tp_[:], axis=X)
        # drop self-comparison (additive -1e30 one-hot), exp with fused row-sum
        nc.vector.tensor_tensor(tt[:], tt[:], adm_sb[:], add)
        ee = sb.tile([rpc, N], f32, tag="ee")
        sums = sb.tile([rpc, 1], f32, tag="sums")
        nc.scalar.activation(
            ee[:], tt[:], AF.Exp, bias=nmx[:], scale=2.0 * TEMP, accum_out=sums[:]
        )
        logt = sb.tile([rpc, 1], f32, tag="logt")
        nc.scalar.activation(logt[:], sums[:], AF.Ln, bias=epsb[:])
        # loss rows = 0.02*(mx' - spos') + log(sum)
        u = sb.tile([rpc, 1], f32, tag="u")
        nc.vector.tensor_tensor(u[:], mx[:], spos[:], sub)
        u2 = sb.tile([rpc, 1], f32, tag="u2")
        nc.vector.tensor_scalar_mul(u2[:], u[:], 2.0 * TEMP)
        lrow = sb.tile([rpc, 1], f32, tag="lrow")
        nc.vector.tensor_tensor(lrow[:], u2[:], logt[:], add)
        nc.sync.dma_start(out, lrow[:])


_NC_CACHE = {}


def _get_nc():
    if "nc" not in _NC_CACHE:
        _NC_CACHE["nc"] = build_nc()
    return _NC_CACHE["nc"]


def make_in_maps(feature1, feature2, n_cores=N_CORES):
    f1 = np.asarray(feature1, dtype=np.float32).reshape(B, -1)
    f2 = np.asarray(feature2, dtype=np.float32).reshape(B, -1)
    contrast = np.concatenate([f1, f2], axis=0)  # (512, K)
    ktot = contrast.shape[1]
    kshard = ktot // n_cores
    rpc = N // n_cores
    sq = np.einsum("ij,ij->i", contrast, contrast, dtype=np.float32)  # (512,)
    ct_f8 = contrast.T.astype(ml_dtypes.float8_e4m3fn)  # (K, 512) transpose+cast
    idx = np.arange(N)
    in_maps = []
    for c in range(n_cores):
        rows = np.arange(rpc) + c * rpc
        adm = np.zeros((rpc, N), np.float32)
        adm[np.arange(rpc), rows] = -1.0e30
        pmask = np.zeros((rpc, N), np.float32)
        pmask[np.arange(rpc), (rows + B) % N] = 1.0
        sqbc = np.tile((-0.5 * sq)[None, :], (rpc, 1)).astype(np.float32)
        sqbc[np.arange(rpc), rows] += sq[rows]
        # pre-swizzled (partition, chunk, col) so each DMA group reads
        # per-partition contiguous bytes instead of 512B strided segments
        sh = np.ascontiguousarray(
            ct_f8[c * kshard : (c + 1) * kshard].reshape(-1, P, N).transpose(1, 0, 2)
        )
        # subtract this core's own fp8-computed gram diagonal before the fp16
        # collective; the exact diagonal is re-added via sqbc. This both keeps
        # the values in fp16 range and cancels the fp8 sum(r^2) diagonal bias.
        shf = sh.astype(np.float32)
        sq8c = np.einsum("pcj,pcj->j", shf, shf, dtype=np.float32)
        dsub = np.zeros((N // P, P, N), np.float32)
        dsub[idx // P, idx % P, idx] = sq8c
        in_maps.append({"ct": sh, "sqb": sqbc, "adm": adm, "pm": pmask, "dsub": dsub})
    return in_maps


def run(feature1, feature2, **spmd_kwargs):
    """Returns (loss_scalar, BassKernelResults)."""
    in_maps = make_in_maps(feature1, feature2)
    nc = _get_nc()
    res = run_bass_kernel_spmd(nc, in_maps, core_ids=list(range(N_CORES)), **spmd_kwargs)
    val = np.float32(
        sum(float(np.asarray(res.results[c]["out"]).sum(dtype=np.float64)) for c in range(N_CORES)) / N
    )
    return np.asarray(val, dtype=np.float32).reshape(()), res


def kernel(feature1, feature2):
    val, _ = run(feature1, feature2)
    return val

